# revision 5
# baseline (speedup 1.0000x reference)
"""Trainium2 Bass kernel for nn_CAGroup3DRoIHead (sparse conv + BN + ELU +
grid pooling + BN), 8-core SPMD.

Sharding: stage 1 (sparse conv) and stage 2 (grid pooling) are both
sharded by pooling cell p (43 cells per core); each core only processes
the unique voxels its cells reference.  Host does all integer index
math and pre-gathers sp_feats rows into a k-major padded table per core
(pure data movement); device does all float compute: 125 matmuls vs W1
(bf16), one SBUF gather (multi-hit planes + stage-2 slot table), a PE
one-hot scatter of multi-hit sums, masked global-BN stats + tiny
AllReduce, BN+ELU, pooling conv vs W2 (bf16, one-hot matmul aggregation
over ROIs), pooled AllReduce, final BN.
"""
import numpy as np
import ml_dtypes

G = 384
HALF = G // 2
SZ = G
SYZ = G * G
SXYZ = G * G * G
VOX = np.float32(0.08)
CK = 2
K = 5
K3 = K ** 3
GN = 7
EPS = 1e-5
C = 128
NCORES = 8
NV = 200000
N = 175616
P = GN ** 3          # 343
B_ROIS = 512
GP = 43              # p-cells per core (core 7: 42 real + 1 pad)
SLOT = 128           # padded slots per p-cell

BF16 = ml_dtypes.bfloat16
_cache = {}


def _pad(n, m):
    return ((int(n) + m - 1) // m) * m


def _wrap16(lst, pad_to):
    lst = np.asarray(lst, np.int64)
    n = len(lst)
    s = (pad_to + 15) // 16
    out = np.zeros((128, s), np.int16)
    padded = np.concatenate([lst, np.zeros(pad_to - n, np.int64)])
    for r in range(8):
        for p in range(16):
            row = padded[p::16]
            out[16 * r + p, :len(row)] = row
    return out


def _host_indices(sp_coords, grid_points):
    """Replicates reference.py's integer index math exactly."""
    sp_coords = np.asarray(sp_coords)
    grid_points = np.asarray(grid_points, np.float32)
    vox = np.clip(np.floor(grid_points[:, 1:4] / VOX).astype(np.int32),
                  -(HALF - 1), HALF - 1).astype(np.int64)
    pos = vox + HALF
    bidx = grid_points[:, 0].astype(np.int64)
    mc = bidx * SXYZ + pos[:, 0] * SYZ + pos[:, 1] * SZ + pos[:, 2]
    unq, unq_inv = np.unique(mc, return_inverse=True)
    Nq = len(unq)
    qb = unq // SXYZ
    qv = np.stack([unq % SXYZ // SYZ, unq % SYZ // SZ, unq % SZ], 1)

    svi = sp_coords[:, 1:4].astype(np.int64) // CK + HALF
    scode = sp_coords[:, 0].astype(np.int64) * SXYZ + svi[:, 0] * SYZ \
        + svi[:, 1] * SZ + svi[:, 2]
    order = np.argsort(scode, kind="stable")
    scodes = scode[order]

    kr = np.arange(-(K // 2), K // 2 + 1)
    offs = np.stack(np.meshgrid(kr, kr, kr, indexing="ij"), -1).reshape(-1, 3)
    hit_rows, hit_ks, hit_sp = [], [], []
    for k in range(K3):
        tvi = qv + offs[k]
        inb = np.all((tvi >= 0) & (tvi < G), axis=1)
        code = qb * SXYZ + tvi[:, 0] * SYZ + tvi[:, 1] * SZ + tvi[:, 2]
        pp = np.clip(np.searchsorted(scodes, code), 0, NV - 1)
        hit = (scodes[pp] == code) & inb
        w = np.nonzero(hit)[0]
        hit_rows.append(w)
        hit_ks.append(np.full(len(w), k, np.int64))
        hit_sp.append(order[pp[w]])
    hit_rows = np.concatenate(hit_rows)   # voxel slot of hit, k-major order
    hit_ks = np.concatenate(hit_ks)
    hit_sp = np.concatenate(hit_sp)
    return Nq, unq_inv, hit_rows, hit_ks, hit_sp


def _shard(Nq, unq_inv, hit_rows, hit_ks, hit_sp):
    """Partition work by pooling cell p; build per-core index tables."""
    pp = np.arange(N) % P
    bb = np.arange(N) // P
    core_slot = np.minimum(pp // GP, NCORES - 1)
    order_v = np.argsort(hit_rows, kind="stable")
    row_of = hit_rows[order_v]
    has_hit = np.zeros(Nq, bool)
    has_hit[hit_rows] = True
    owner = np.full(Nq, NCORES, np.int64)
    for c in range(NCORES - 1, -1, -1):
        owner[np.unique(unq_inv[core_slot == c])] = c

    raw = []
    for c in range(NCORES):
        slots_c = np.nonzero((core_slot == c) & has_hit[unq_inv])[0]
        vox_c = np.unique(unq_inv[slots_c])          # sorted, all have hits
        a = np.searchsorted(row_of, vox_c, "left")
        b = np.searchsorted(row_of, vox_c, "right")
        cnt = b - a
        all_h = np.concatenate([order_v[x:y] for x, y in zip(a, b)]) \
            if len(vox_c) else np.zeros(0, np.int64)
        raw.append((slots_c, vox_c, a, cnt, all_h))

    # shared padded sizes (max over cores)
    KW = 4
    maxcnt = 2
    max_multi = 1
    for c in range(NCORES):
        slots_c, vox_c, a, cnt, all_h = raw[c]
        kc = np.bincount(hit_ks[all_h], minlength=K3)
        KW = max(KW, _pad(kc.max(), 4))
        maxcnt = max(maxcnt, int(cnt.max()))
        max_multi = max(max_multi, int((cnt >= 2).sum()))
    NYW = K3 * KW
    MW = _pad(max_multi, 128)                        # scatter-source chunks
    NSC = MW // 128
    NRND = maxcnt - 1
    RWS = []
    for j in range(1, maxcnt):
        m = max(max(int((r[3] > j).sum()) for r in raw), 1)
        RWS.append(_pad(m, 4))
    GRW = _pad(sum(RWS), 16)
    ZC = NYW                                         # zero col, Y-relative
    XGW = GP * SLOT
    GB = MW + GRW                                    # xg offset in gout
    GOW = GB + XGW                                   # gather output width

    cores = []
    for c in range(NCORES):
        slots_c, vox_c, a, cnt, all_h = raw[c]
        ks_h = hit_ks[all_h]
        sp_h = hit_sp[all_h]
        ord_k = np.argsort(ks_h, kind="stable")
        ks_sorted = ks_h[ord_k]
        kcounts = np.bincount(ks_sorted, minlength=K3)
        kstart = np.concatenate([[0], np.cumsum(kcounts)])[:-1]
        rank = np.arange(len(ks_sorted)) - kstart[ks_sorted]
        col_of_hit = np.empty(len(all_h), np.int64)
        col_of_hit[ord_k] = ks_sorted * KW + rank
        starts = np.concatenate([[0], np.cumsum(cnt)])
        first_col = col_of_hit[starts[:-1]]

        multi_mask = cnt >= 2
        m_idx = np.nonzero(multi_mask)[0]
        m_ord = np.argsort(-cnt[multi_mask], kind="stable")
        multi_vloc = m_idx[m_ord]                    # count-desc local vox ids
        n_multi = len(multi_vloc)
        mcnt = cnt[multi_vloc]
        pos_in_accr = np.full(len(vox_c), -1, np.int64)
        pos_in_accr[multi_vloc] = np.arange(n_multi)

        # gather idx: [P0 hit1 of multis | planes hits 2..6 | xg slots]
        gidx = np.full(GOW, ZC, np.int64)
        gidx[:n_multi] = first_col[multi_vloc]
        off = MW
        for j in range(1, maxcnt):
            sel = multi_vloc[mcnt > j]               # prefix (count-desc)
            nj = len(sel)
            gidx[off:off + nj] = col_of_hit[starts[sel] + j]
            off += RWS[j - 1]

        bv = np.full(XGW, 600.0, np.float32)
        selm = np.zeros((MW, XGW), np.float32)       # scatter one-hot
        msk = np.zeros(XGW, np.float32)              # owned stats mask
        own = owner[vox_c] == c
        seen = np.zeros(len(vox_c), bool)
        vox_of_slot = unq_inv[slots_c]
        p_of_slot = pp[slots_c]
        for lp in range(GP):
            p = c * GP + lp
            if p >= P:
                continue
            m = p_of_slot == p
            sl = slots_c[m]
            n = len(sl)
            assert n <= SLOT
            vl = np.searchsorted(vox_c, vox_of_slot[m])
            dst = GB + lp * SLOT + np.arange(n)
            is_m = pos_in_accr[vl] >= 0
            gidx[dst[~is_m]] = first_col[vl[~is_m]]  # singles: direct col
            selm[pos_in_accr[vl[is_m]], dst[is_m] - GB] = 1.0
            newly = own[vl] & ~seen[vl]
            msk[dst[newly] - GB] = 1.0
            seen[vl[newly]] = True
            bv[lp * SLOT: lp * SLOT + n] = bb[sl]
        cores.append(dict(fcols=col_of_hit, frows=sp_h, gidx=gidx, bv=bv,
                          selm=selm, msk=msk, n_multi=n_multi))
    sizes = dict(KW=KW, NYW=NYW, MW=MW, NSC=NSC, NRND=NRND, RWS=tuple(RWS),
                 GRW=GRW, ZC=ZC, XGW=XGW, GB=GB, GOW=GOW, Nq=Nq)
    return cores, sizes


def _compile(S):
    import concourse.bass as bass
    import concourse.bacc as bacc
    import concourse.tile as tile
    from concourse import mybir
    from concourse.masks import make_identity

    f32 = mybir.dt.float32
    bf16 = mybir.dt.bfloat16
    i16 = mybir.dt.int16
    AF = mybir.ActivationFunctionType
    OP = mybir.AluOpType
    AX = mybir.AxisListType

    KW, NYW, MW, NSC, NRND = S["KW"], S["NYW"], S["MW"], S["NSC"], S["NRND"]
    RWS, GRW, XGW, GB, GOW = S["RWS"], S["GRW"], S["XGW"], S["GB"], S["GOW"]
    Nq = float(S["Nq"])
    XTW = NYW + 128
    NCH = (NYW + 511) // 512
    NWRM = 4                                         # warm-up dummy gathers

    nc = bacc.Bacc("TRN2", target_bir_lowering=False, debug=False,
                   num_devices=NCORES, num_swdge_queues=1)
    f_p = nc.declare_dram_parameter("f", [C, NYW], bf16, isOutput=False)
    w1 = nc.declare_dram_parameter("w1", [C, K3 * C], bf16, isOutput=False)
    w2 = nc.declare_dram_parameter("w2", [C, GP * C], bf16, isOutput=False)
    ws = nc.declare_dram_parameter("ws", [C, C], f32, isOutput=False)
    sel = nc.declare_dram_parameter("sel", [C, NSC * XGW], bf16,
                                    isOutput=False)
    msk = nc.declare_dram_parameter("msk", [C, XGW], f32, isOutput=False)
    g1 = nc.declare_dram_parameter("g1", [C, 1], f32, isOutput=False)
    b1 = nc.declare_dram_parameter("b1", [C, 1], f32, isOutput=False)
    g2 = nc.declare_dram_parameter("g2", [C, 1], f32, isOutput=False)
    b2 = nc.declare_dram_parameter("b2", [C, 1], f32, isOutput=False)
    gst = nc.declare_dram_parameter("gst", [128, GOW // 16], i16,
                                    isOutput=False)
    wut = nc.declare_dram_parameter("wut", [128, 1], i16, isOutput=False)
    bvt = nc.declare_dram_parameter("bvt", [128, GP], f32, isOutput=False)
    iot = nc.declare_dram_parameter("iot", [128, 512], f32, isOutput=False)
    out = nc.declare_dram_parameter("out", [C, B_ROIS], f32, isOutput=True)
    cc1i = nc.dram_tensor("cc1i", [C, 2], f32)
    cc1o = nc.dram_tensor("cc1o", [C, 2], f32)
    cc2i = nc.dram_tensor("cc2i", [C, B_ROIS], f32)
    cc2o = nc.dram_tensor("cc2o", [C, B_ROIS], f32)

    with tile.TileContext(nc) as tc:
        with (
            tc.tile_pool(name="sm", bufs=2) as sm,
            tc.tile_pool(name="big", bufs=1) as big,
            tc.tile_pool(name="pa", bufs=2, space="PSUM") as pa,
            tc.tile_pool(name="pb", bufs=2, space="PSUM") as pb,
            tc.tile_pool(name="pc", bufs=1, space="PSUM") as pc,
        ):
            ident = big.tile([128, 128], f32)
            make_identity(nc, ident[:])
            epst = big.tile([128, 1], f32)
            nc.vector.memset(epst[:], EPS)
            gs_t = big.tile([128, GOW // 16], i16)
            nc.sync.dma_start(out=gs_t[:], in_=gst[:])
            wu_t = big.tile([128, 1], i16)
            nc.sync.dma_start(out=wu_t[:], in_=wut[:])
            bv_t = big.tile([128, GP], f32)
            nc.sync.dma_start(out=bv_t[:], in_=bvt[:])
            io_t = big.tile([128, 512], f32)
            nc.sync.dma_start(out=io_t[:], in_=iot[:])
            g1t = big.tile([128, 1], f32); nc.sync.dma_start(out=g1t[:], in_=g1[:])
            b1t = big.tile([128, 1], f32); nc.sync.dma_start(out=b1t[:], in_=b1[:])
            g2t = big.tile([128, 1], f32); nc.sync.dma_start(out=g2t[:], in_=g2[:])
            b2t = big.tile([128, 1], f32); nc.sync.dma_start(out=b2t[:], in_=b2[:])
            wst = big.tile([128, C], f32)
            nc.sync.dma_start(out=wst[:], in_=ws[:])
            selt = big.tile([128, NSC * XGW], bf16)
            nc.sync.dma_start(out=selt[:], in_=sel[:])
            mskt = big.tile([128, XGW], f32)
            nc.sync.dma_start(out=mskt[:], in_=msk[:])

            ft = big.tile([128, NYW], bf16)
            nc.sync.dma_start(out=ft[:], in_=f_p[:])
            w1t = big.tile([128, K3 * C], bf16)
            HW1 = (K3 // 2) * C
            nc.sync.dma_start(out=w1t[:, :HW1], in_=w1[:, :HW1])
            nc.sync.dma_start(out=w1t[:, HW1:], in_=w1[:, HW1:K3 * C])
            w2t = big.tile([128, GP * C], bf16)
            nc.sync.dma_start(out=w2t[:], in_=w2[:])

            # ---------- stage 1: 125 matmuls vs W1 ----------
            xtab = big.tile([128, XTW], f32)
            nc.vector.memset(xtab[:, NYW:XTW], 0.0)
            wrm = big.tile([128, 16 * (NWRM + 1)], f32)
            nc.gpsimd.ap_gather(          # keep DSP pool warm (wake #0)
                out_ap=wrm[:, 0:16].rearrange("p (n u) -> p n u", u=1),
                in_ap=xtab[:, NYW:NYW + 128].rearrange("p (n u) -> p n u", u=1),
                idxs_ap=wu_t[:], channels=128, num_elems=128, d=1, num_idxs=16)
            for ch in range(NCH):
                c0, c1 = ch * 512, min(ch * 512 + 512, NYW)
                yp = pa.tile([128, 512], f32, tag="yp")
                for k in range(c0 // KW, (c1 + KW - 1) // KW):
                    a = max(k * KW, c0)
                    b = min((k + 1) * KW, c1)
                    if a >= b:
                        continue
                    nc.tensor.matmul(out=yp[:, a - c0:b - c0],
                                     lhsT=w1t[:, k * C:(k + 1) * C],
                                     rhs=ft[:, a:b], start=True, stop=True)
                nc.scalar.activation(out=xtab[:, c0:c1],
                                     in_=yp[:, :c1 - c0], func=AF.Copy)
                if ch % 2 == 1 and ch // 2 < NWRM:   # warm-up drumbeat
                    w = ch // 2 + 1
                    nc.gpsimd.ap_gather(
                        out_ap=wrm[:, w * 16:w * 16 + 16].rearrange(
                            "p (n u) -> p n u", u=1),
                        in_ap=xtab[:, c0:c0 + 128].rearrange(
                            "p (n u) -> p n u", u=1),
                        idxs_ap=wu_t[:], channels=128, num_elems=128, d=1,
                        num_idxs=16)

            # ---------- one gather: multi planes + stage-2 slot table ------
            gout = big.tile([128, GOW], f32)
            nc.gpsimd.ap_gather(
                out_ap=gout[:, 0:GOW].rearrange("p (n u) -> p n u", u=1),
                in_ap=xtab[:, 0:XTW].rearrange("p (n u) -> p n u", u=1),
                idxs_ap=gs_t[:], channels=128, num_elems=XTW, d=1,
                num_idxs=GOW)
            off = MW
            for j in range(NRND):
                rw = RWS[j]
                nc.vector.tensor_tensor(out=gout[:, :rw], in0=gout[:, :rw],
                                        in1=gout[:, off:off + rw], op=OP.add)
                off += rw

            # ---------- PE scatter of multi sums into slot table ----------
            acct = big.tile([128, MW], bf16)
            for j in range(NSC):
                tp = pa.tile([128, 128], f32, tag="yp")
                nc.tensor.transpose(out=tp[:],
                                    in_=gout[:, j * 128:(j + 1) * 128],
                                    identity=ident[:])
                nc.vector.tensor_copy(out=acct[:, j * 128:(j + 1) * 128],
                                      in_=tp[:])
            for c0 in range(0, XGW, 512):
                c1 = min(c0 + 512, XGW)
                psc = pb.tile([128, 512], f32, tag="psc")
                for j in range(NSC):
                    nc.tensor.matmul(out=psc[:, :c1 - c0],
                                     lhsT=acct[:, j * 128:(j + 1) * 128],
                                     rhs=selt[:, j * XGW + c0:j * XGW + c1],
                                     start=(j == 0), stop=(j == NSC - 1))
                nc.vector.tensor_tensor(out=gout[:, GB + c0:GB + c1],
                                        in0=gout[:, GB + c0:GB + c1],
                                        in1=psc[:, :c1 - c0], op=OP.add)

            # ---------- masked BN1 stats + AllReduce ----------
            sx = gout[:, GB:GB + XGW]
            r_ = big.tile([128, XGW], f32)
            nc.vector.tensor_tensor(out=r_[:], in0=sx, in1=mskt[:],
                                    op=OP.mult)
            st = big.tile([128, 2], f32)
            nc.vector.reduce_sum(out=st[:, 0:1], in_=r_[:], axis=AX.X)
            nc.scalar.activation(out=r_[:], in_=r_[:], func=AF.Square,
                                 accum_out=st[:, 1:2])
            nc.sync.dma_start(out=cc1i[:], in_=st[:])
            nc.gpsimd.collective_compute(
                "AllReduce", OP.add, replica_groups=[list(range(NCORES))],
                ins=[cc1i[:]], outs=[cc1o[:]])
            stg = big.tile([128, 2], f32)
            nc.sync.dma_start(out=stg[:], in_=cc1o[:])

            # ---------- BN1 constants ----------
            mean = big.tile([128, 1], f32)
            nc.vector.tensor_scalar_mul(out=mean[:], in0=stg[:, 0:1],
                                        scalar1=1.0 / Nq)
            var = big.tile([128, 1], f32)
            nc.vector.tensor_scalar_mul(out=var[:], in0=stg[:, 1:2],
                                        scalar1=1.0 / Nq)
            m2 = big.tile([128, 1], f32)
            nc.vector.tensor_tensor(out=m2[:], in0=mean[:], in1=mean[:],
                                    op=OP.mult)
            nc.vector.tensor_tensor(out=var[:], in0=var[:], in1=m2[:],
                                    op=OP.subtract)
            sd = big.tile([128, 1], f32)
            nc.scalar.activation(out=sd[:], in_=var[:], func=AF.Sqrt,
                                 bias=epst[:, :1])
            rs = big.tile([128, 1], f32)
            nc.vector.reciprocal(out=rs[:], in_=sd[:])
            rsg = big.tile([128, 1], f32)
            nc.vector.tensor_tensor(out=rsg[:], in0=rs[:], in1=g1t[:],
                                    op=OP.mult)
            shift = big.tile([128, 1], f32)
            nc.vector.tensor_tensor(out=shift[:], in0=mean[:], in1=rsg[:],
                                    op=OP.mult)
            nc.vector.tensor_tensor(out=shift[:], in0=b1t[:], in1=shift[:],
                                    op=OP.subtract)
            xz = big.tile([128, 1], f32)
            t1 = big.tile([128, 1], f32)
            nc.scalar.activation(out=xz[:], in_=shift[:], func=AF.Relu)
            nc.vector.tensor_scalar_min(out=t1[:], in0=shift[:], scalar1=0.0)
            nc.scalar.activation(out=t1[:], in_=t1[:], func=AF.Exp)
            nc.vector.tensor_tensor(out=xz[:], in0=xz[:], in1=t1[:], op=OP.add)
            nc.vector.tensor_scalar_add(out=xz[:], in0=xz[:], scalar1=-1.0)
            ccol = big.tile([128, 1], f32)
            nc.vector.tensor_scalar(out=ccol[:], in0=xz[:], scalar1=-1.0,
                                    scalar2=-1.0, op0=OP.mult, op1=OP.add)

            # ---------- BN + ELU on stage-2 cols, minus xz ----------
            nc.vector.tensor_scalar(out=sx, in0=sx, scalar1=rsg[:, :1],
                                    scalar2=shift[:, :1], op0=OP.mult,
                                    op1=OP.add)
            nc.scalar.activation(out=r_[:], in_=sx, func=AF.Relu)
            nc.vector.tensor_scalar_min(out=sx, in0=sx, scalar1=0.0)
            nc.scalar.activation(out=sx, in_=sx, func=AF.Exp)
            nc.vector.tensor_tensor(out=sx, in0=sx, in1=r_[:], op=OP.add)
            nc.vector.tensor_scalar(out=sx, in0=sx, scalar1=ccol[:, :1],
                                    scalar2=None, op0=OP.add)
            sxb = big.tile([128, XGW], bf16)
            nc.vector.tensor_copy(out=sxb[:], in_=sx)

            # ---------- pooling conv: corrections + one-hot aggregation ----
            pool_p = pc.tile([128, 512], f32, tag="pool")
            for q0 in range(0, GP, 4):
                qn = min(4, GP - q0)
                cp = pb.tile([128, 512], f32, tag="psc")
                for lp in range(q0, q0 + qn):
                    nc.tensor.matmul(
                        out=cp[:, (lp - q0) * 128:(lp - q0 + 1) * 128],
                        lhsT=sxb[:, lp * SLOT:(lp + 1) * SLOT],
                        rhs=w2t[:, lp * C:(lp + 1) * C],
                        start=True, stop=True)
                cbf = sm.tile([128, 512], bf16, tag="cbf")
                nc.vector.tensor_copy(out=cbf[:, :qn * 128],
                                      in_=cp[:, :qn * 128])
                for lp in range(q0, q0 + qn):
                    oh = sm.tile([128, 512], bf16, tag="oh")
                    nc.vector.tensor_tensor(
                        out=oh[:],
                        in0=bv_t[:, lp:lp + 1].to_broadcast([128, 512]),
                        in1=io_t[:], op=OP.is_equal)
                    nc.tensor.matmul(
                        out=pool_p[:],
                        lhsT=cbf[:, (lp - q0) * 128:(lp - q0 + 1) * 128],
                        rhs=oh[:], start=(lp == 0), stop=(lp == GP - 1))
            basep = pa.tile([128, 1], f32, tag="yp")
            nc.tensor.matmul(out=basep[:], lhsT=wst[:], rhs=xz[:, :1],
                             start=True, stop=True)
            base = big.tile([128, 1], f32)
            nc.vector.tensor_copy(out=base[:], in_=basep[:])
            pl = big.tile([128, 512], f32)
            nc.vector.tensor_copy(out=pl[:], in_=pool_p[:])
            nc.vector.tensor_scalar(out=pl[:], in0=pl[:], scalar1=base[:, :1],
                                    scalar2=None, op0=OP.add)

            # ---------- pooled AllReduce + final BN ----------
            nc.sync.dma_start(out=cc2i[:], in_=pl[:])
            nc.gpsimd.collective_compute(
                "AllReduce", OP.add, replica_groups=[list(range(NCORES))],
                ins=[cc2i[:]], outs=[cc2o[:]])
            pf = big.tile([128, 512], f32)
            nc.sync.dma_start(out=pf[:], in_=cc2o[:])
            mn2 = big.tile([128, 1], f32)
            nc.vector.reduce_sum(out=mn2[:], in_=pf[:], axis=AX.X)
            nc.vector.tensor_scalar_mul(out=mn2[:], in0=mn2[:],
                                        scalar1=1.0 / B_ROIS)
            sq2 = big.tile([128, 1], f32)
            scr2 = big.tile([128, 512], f32)
            nc.scalar.activation(out=scr2[:], in_=pf[:], func=AF.Square,
                                 accum_out=sq2[:])
            nc.vector.tensor_scalar_mul(out=sq2[:], in0=sq2[:],
                                        scalar1=1.0 / B_ROIS)
            m22 = big.tile([128, 1], f32)
            nc.vector.tensor_tensor(out=m22[:], in0=mn2[:], in1=mn2[:],
                                    op=OP.mult)
            nc.vector.tensor_tensor(out=sq2[:], in0=sq2[:], in1=m22[:],
                                    op=OP.subtract)
            sd2 = big.tile([128, 1], f32)
            nc.scalar.activation(out=sd2[:], in_=sq2[:], func=AF.Sqrt,
                                 bias=epst[:, :1])
            rs2 = big.tile([128, 1], f32)
            nc.vector.reciprocal(out=rs2[:], in_=sd2[:])
            rsg2 = big.tile([128, 1], f32)
            nc.vector.tensor_tensor(out=rsg2[:], in0=rs2[:], in1=g2t[:],
                                    op=OP.mult)
            sh2 = big.tile([128, 1], f32)
            nc.vector.tensor_tensor(out=sh2[:], in0=mn2[:], in1=rsg2[:],
                                    op=OP.mult)
            nc.vector.tensor_tensor(out=sh2[:], in0=b2t[:], in1=sh2[:],
                                    op=OP.subtract)
            nc.vector.tensor_scalar(out=pf[:], in0=pf[:], scalar1=rsg2[:, :1],
                                    scalar2=sh2[:, :1], op0=OP.mult,
                                    op1=OP.add)
            nc.sync.dma_start(out=out[:], in_=pf[:])

    nc.compile()
    return nc


def _build_inputs(cores, S, sp_feats, W1, W2, gamma1, beta1, gamma2, beta2):
    NYW, GOW, NSC, XGW = S["NYW"], S["GOW"], S["NSC"], S["XGW"]
    W1t = np.ascontiguousarray(
        W1.transpose(1, 0, 2).reshape(C, K3 * C)).astype(BF16)
    base_in = {
        "w1": W1t,
        "g1": gamma1.reshape(C, 1), "b1": beta1.reshape(C, 1),
        "g2": gamma2.reshape(C, 1), "b2": beta2.reshape(C, 1),
        "iot": np.broadcast_to(np.arange(512, dtype=np.float32),
                               (128, 512)).copy(),
        "wut": np.zeros((128, 1), np.int16),
    }
    in_maps = []
    for c in range(NCORES):
        L = cores[c]
        m = dict(base_in)
        F = np.zeros((C, NYW), np.float32)
        F[:, L["fcols"]] = sp_feats[L["frows"]].T
        m["f"] = F.astype(BF16)
        w2l = np.zeros((GP, C, C), np.float32)
        p0 = c * GP
        nreal = max(0, min(GP, P - p0))
        w2l[:nreal] = W2[p0:p0 + nreal]
        m["ws"] = np.ascontiguousarray(w2l.sum(0))
        m["w2"] = np.ascontiguousarray(
            w2l.transpose(1, 0, 2).reshape(C, GP * C)).astype(BF16)
        m["gst"] = _wrap16(L["gidx"], GOW)
        # selm [MW, XGW] -> [128, NSC*XGW] (source chunk j in partitions)
        selw = np.zeros((128, NSC * XGW), np.float32)
        for j in range(NSC):
            selw[:, j * XGW:(j + 1) * XGW] = L["selm"][j * 128:(j + 1) * 128]
        m["sel"] = selw.astype(BF16)
        m["msk"] = np.broadcast_to(L["msk"], (C, XGW)).copy()
        bvw = np.zeros((128, GP), np.float32)
        for lp in range(GP):
            bvw[:, lp] = L["bv"][lp * SLOT:(lp + 1) * SLOT]
        m["bvt"] = bvw
        in_maps.append(m)
    return in_maps


def kernel(**inputs):
    sp_coords = np.asarray(inputs["sp_coords"])
    sp_feats = np.asarray(inputs["sp_feats"], np.float32)
    grid_points = np.asarray(inputs["grid_points"], np.float32)
    W1 = np.asarray(inputs["W1"], np.float32)
    gamma1 = np.asarray(inputs["gamma1"], np.float32)
    beta1 = np.asarray(inputs["beta1"], np.float32)
    W2 = np.asarray(inputs["W2"], np.float32)
    gamma2 = np.asarray(inputs["gamma2"], np.float32)
    beta2 = np.asarray(inputs["beta2"], np.float32)

    Nq, unq_inv, hit_rows, hit_ks, hit_sp = _host_indices(sp_coords,
                                                          grid_points)
    cores, S = _shard(Nq, unq_inv, hit_rows, hit_ks, hit_sp)

    key = tuple(sorted((k, v) for k, v in S.items() if k != "RWS")) \
        + S["RWS"]
    if key not in _cache:
        _cache.clear()
        _cache[key] = _compile(S)
    nc = _cache[key]

    in_maps = _build_inputs(cores, S, sp_feats, W1, W2, gamma1, beta1,
                            gamma2, beta2)

    import os
    from concourse.bass_utils import run_bass_kernel_spmd
    trace = os.environ.get("KERNEL_TRACE", "0") == "1"
    if trace:
        try:
            import ntff_hook
            ntff_hook.install()
        except Exception:
            trace = False
    res = run_bass_kernel_spmd(nc, in_maps, list(range(NCORES)), trace=trace)
    if trace and res.exec_time_ns:
        print("HW exec time: %d ns" % res.exec_time_ns)
    return np.ascontiguousarray(
        np.asarray(res.results[0]["out"], np.float32).T)


# revision 6
# speedup vs baseline: 1.0564x; 1.0564x over previous
"""Trainium2 Bass kernel for nn_CAGroup3DRoIHead (sparse conv + BN + ELU +
grid pooling + BN), 8-core SPMD.

Sharding: stage 1 (sparse conv) and stage 2 (grid pooling) are both
sharded by pooling cell p (43 cells per core); each core only processes
the unique voxels its cells reference.  Host does all integer index
math and pre-gathers sp_feats rows into a k-major padded table per core
(pure data movement); device does all float compute: 125 matmuls vs W1
(bf16), one SBUF gather (multi-hit planes + stage-2 slot table), a PE
one-hot scatter of multi-hit sums, masked global-BN stats + tiny
AllReduce, BN+ELU, pooling conv vs W2 (bf16, one-hot matmul aggregation
over ROIs), pooled AllReduce, final BN.
"""
import numpy as np
import ml_dtypes

G = 384
HALF = G // 2
SZ = G
SYZ = G * G
SXYZ = G * G * G
VOX = np.float32(0.08)
CK = 2
K = 5
K3 = K ** 3
GN = 7
EPS = 1e-5
C = 128
NCORES = 8
NV = 200000
N = 175616
P = GN ** 3          # 343
B_ROIS = 512
GP = 43              # p-cells per core (core 7: 42 real + 1 pad)
SLOT = 128           # padded slots per p-cell

BF16 = ml_dtypes.bfloat16
_cache = {}


def _pad(n, m):
    return ((int(n) + m - 1) // m) * m


def _wrap16(lst, pad_to):
    lst = np.asarray(lst, np.int64)
    n = len(lst)
    s = (pad_to + 15) // 16
    out = np.zeros((128, s), np.int16)
    padded = np.concatenate([lst, np.zeros(pad_to - n, np.int64)])
    for r in range(8):
        for p in range(16):
            row = padded[p::16]
            out[16 * r + p, :len(row)] = row
    return out


def _host_indices(sp_coords, grid_points):
    """Replicates reference.py's integer index math exactly."""
    sp_coords = np.asarray(sp_coords)
    grid_points = np.asarray(grid_points, np.float32)
    vox = np.clip(np.floor(grid_points[:, 1:4] / VOX).astype(np.int32),
                  -(HALF - 1), HALF - 1).astype(np.int64)
    pos = vox + HALF
    bidx = grid_points[:, 0].astype(np.int64)
    mc = bidx * SXYZ + pos[:, 0] * SYZ + pos[:, 1] * SZ + pos[:, 2]
    unq, unq_inv = np.unique(mc, return_inverse=True)
    Nq = len(unq)
    qb = unq // SXYZ
    qv = np.stack([unq % SXYZ // SYZ, unq % SYZ // SZ, unq % SZ], 1)

    svi = sp_coords[:, 1:4].astype(np.int64) // CK + HALF
    scode = sp_coords[:, 0].astype(np.int64) * SXYZ + svi[:, 0] * SYZ \
        + svi[:, 1] * SZ + svi[:, 2]
    order = np.argsort(scode, kind="stable")
    scodes = scode[order]

    kr = np.arange(-(K // 2), K // 2 + 1)
    offs = np.stack(np.meshgrid(kr, kr, kr, indexing="ij"), -1).reshape(-1, 3)
    hit_rows, hit_ks, hit_sp = [], [], []
    for k in range(K3):
        tvi = qv + offs[k]
        inb = np.all((tvi >= 0) & (tvi < G), axis=1)
        code = qb * SXYZ + tvi[:, 0] * SYZ + tvi[:, 1] * SZ + tvi[:, 2]
        pp = np.clip(np.searchsorted(scodes, code), 0, NV - 1)
        hit = (scodes[pp] == code) & inb
        w = np.nonzero(hit)[0]
        hit_rows.append(w)
        hit_ks.append(np.full(len(w), k, np.int64))
        hit_sp.append(order[pp[w]])
    hit_rows = np.concatenate(hit_rows)   # voxel slot of hit, k-major order
    hit_ks = np.concatenate(hit_ks)
    hit_sp = np.concatenate(hit_sp)
    return Nq, unq_inv, hit_rows, hit_ks, hit_sp


def _shard(Nq, unq_inv, hit_rows, hit_ks, hit_sp):
    """Partition work by pooling cell p; build per-core index tables."""
    pp = np.arange(N) % P
    bb = np.arange(N) // P
    core_slot = np.minimum(pp // GP, NCORES - 1)
    order_v = np.argsort(hit_rows, kind="stable")
    row_of = hit_rows[order_v]
    has_hit = np.zeros(Nq, bool)
    has_hit[hit_rows] = True
    owner = np.full(Nq, NCORES, np.int64)
    for c in range(NCORES - 1, -1, -1):
        owner[np.unique(unq_inv[core_slot == c])] = c

    raw = []
    for c in range(NCORES):
        slots_c = np.nonzero((core_slot == c) & has_hit[unq_inv])[0]
        vox_c = np.unique(unq_inv[slots_c])          # sorted, all have hits
        a = np.searchsorted(row_of, vox_c, "left")
        b = np.searchsorted(row_of, vox_c, "right")
        cnt = b - a
        all_h = np.concatenate([order_v[x:y] for x, y in zip(a, b)]) \
            if len(vox_c) else np.zeros(0, np.int64)
        raw.append((slots_c, vox_c, a, cnt, all_h))

    # shared padded sizes (max over cores)
    KW = 4
    maxcnt = 2
    max_multi = 1
    for c in range(NCORES):
        slots_c, vox_c, a, cnt, all_h = raw[c]
        kc = np.bincount(hit_ks[all_h], minlength=K3)
        KW = max(KW, _pad(kc.max(), 4))
        maxcnt = max(maxcnt, int(cnt.max()))
        max_multi = max(max_multi, int((cnt >= 2).sum()))
    NYW = K3 * KW
    MW = _pad(max_multi, 128)                        # scatter-source chunks
    NSC = MW // 128
    NRND = maxcnt - 1
    RWS = []
    for j in range(1, maxcnt):
        m = max(max(int((r[3] > j).sum()) for r in raw), 1)
        RWS.append(_pad(m, 4))
    GRW = _pad(sum(RWS), 16)
    ZC = NYW                                         # zero col, Y-relative
    XGW = GP * SLOT
    GB = MW + GRW                                    # xg offset in gout
    GOW = GB + XGW                                   # gather output width

    cores = []
    for c in range(NCORES):
        slots_c, vox_c, a, cnt, all_h = raw[c]
        ks_h = hit_ks[all_h]
        sp_h = hit_sp[all_h]
        ord_k = np.argsort(ks_h, kind="stable")
        ks_sorted = ks_h[ord_k]
        kcounts = np.bincount(ks_sorted, minlength=K3)
        kstart = np.concatenate([[0], np.cumsum(kcounts)])[:-1]
        rank = np.arange(len(ks_sorted)) - kstart[ks_sorted]
        col_of_hit = np.empty(len(all_h), np.int64)
        col_of_hit[ord_k] = ks_sorted * KW + rank
        starts = np.concatenate([[0], np.cumsum(cnt)])
        first_col = col_of_hit[starts[:-1]]

        multi_mask = cnt >= 2
        m_idx = np.nonzero(multi_mask)[0]
        m_ord = np.argsort(-cnt[multi_mask], kind="stable")
        multi_vloc = m_idx[m_ord]                    # count-desc local vox ids
        n_multi = len(multi_vloc)
        mcnt = cnt[multi_vloc]
        pos_in_accr = np.full(len(vox_c), -1, np.int64)
        pos_in_accr[multi_vloc] = np.arange(n_multi)

        # gather idx: [P0 hit1 of multis | planes hits 2..6 | xg slots]
        gidx = np.full(GOW, ZC, np.int64)
        gidx[:n_multi] = first_col[multi_vloc]
        off = MW
        for j in range(1, maxcnt):
            sel = multi_vloc[mcnt > j]               # prefix (count-desc)
            nj = len(sel)
            gidx[off:off + nj] = col_of_hit[starts[sel] + j]
            off += RWS[j - 1]

        bv = np.full(XGW, 600.0, np.float32)
        selm = np.zeros((MW, XGW), np.float32)       # scatter one-hot
        msk = np.zeros(XGW, np.float32)              # owned stats mask
        own = owner[vox_c] == c
        seen = np.zeros(len(vox_c), bool)
        vox_of_slot = unq_inv[slots_c]
        p_of_slot = pp[slots_c]
        for lp in range(GP):
            p = c * GP + lp
            if p >= P:
                continue
            m = p_of_slot == p
            sl = slots_c[m]
            n = len(sl)
            assert n <= SLOT
            vl = np.searchsorted(vox_c, vox_of_slot[m])
            dst = GB + lp * SLOT + np.arange(n)
            is_m = pos_in_accr[vl] >= 0
            gidx[dst[~is_m]] = first_col[vl[~is_m]]  # singles: direct col
            selm[pos_in_accr[vl[is_m]], dst[is_m] - GB] = 1.0
            newly = own[vl] & ~seen[vl]
            msk[dst[newly] - GB] = 1.0
            seen[vl[newly]] = True
            bv[lp * SLOT: lp * SLOT + n] = bb[sl]
        cores.append(dict(fcols=col_of_hit, frows=sp_h, gidx=gidx, bv=bv,
                          selm=selm, msk=msk, n_multi=n_multi))
    sizes = dict(KW=KW, NYW=NYW, MW=MW, NSC=NSC, NRND=NRND, RWS=tuple(RWS),
                 GRW=GRW, ZC=ZC, XGW=XGW, GB=GB, GOW=GOW, Nq=Nq)
    return cores, sizes


def _compile(S):
    import concourse.bass as bass
    import concourse.bacc as bacc
    import concourse.tile as tile
    from concourse import mybir
    from concourse.masks import make_identity

    f32 = mybir.dt.float32
    bf16 = mybir.dt.bfloat16
    i16 = mybir.dt.int16
    AF = mybir.ActivationFunctionType
    OP = mybir.AluOpType
    AX = mybir.AxisListType

    KW, NYW, MW, NSC, NRND = S["KW"], S["NYW"], S["MW"], S["NSC"], S["NRND"]
    RWS, GRW, XGW, GB, GOW = S["RWS"], S["GRW"], S["XGW"], S["GB"], S["GOW"]
    Nq = float(S["Nq"])
    XTW = NYW + 128
    NCH = (NYW + 511) // 512
    NWRM = 4                                         # warm-up dummy gathers

    nc = bacc.Bacc("TRN2", target_bir_lowering=False, debug=False,
                   num_devices=NCORES, num_swdge_queues=1)
    f_p = nc.declare_dram_parameter("f", [C, NYW], bf16, isOutput=False)
    w1 = nc.declare_dram_parameter("w1", [C, K3 * C], bf16, isOutput=False)
    w2 = nc.declare_dram_parameter("w2", [C, GP * C], bf16, isOutput=False)
    ws = nc.declare_dram_parameter("ws", [C, C], f32, isOutput=False)
    sel = nc.declare_dram_parameter("sel", [C, NSC * XGW], bf16,
                                    isOutput=False)
    msk = nc.declare_dram_parameter("msk", [C, XGW], f32, isOutput=False)
    g1 = nc.declare_dram_parameter("g1", [C, 1], f32, isOutput=False)
    b1 = nc.declare_dram_parameter("b1", [C, 1], f32, isOutput=False)
    g2 = nc.declare_dram_parameter("g2", [C, 1], f32, isOutput=False)
    b2 = nc.declare_dram_parameter("b2", [C, 1], f32, isOutput=False)
    gst = nc.declare_dram_parameter("gst", [128, GOW // 16], i16,
                                    isOutput=False)
    wut = nc.declare_dram_parameter("wut", [128, 1], i16, isOutput=False)
    bvt = nc.declare_dram_parameter("bvt", [128, GP], f32, isOutput=False)
    iot = nc.declare_dram_parameter("iot", [128, 512], f32, isOutput=False)
    out = nc.declare_dram_parameter("out", [C, B_ROIS], f32, isOutput=True)
    cc1i = nc.dram_tensor("cc1i", [C, 2], f32)
    cc1o = nc.dram_tensor("cc1o", [C, 2], f32)
    cc2i = nc.dram_tensor("cc2i", [C, B_ROIS], f32)
    cc2o = nc.dram_tensor("cc2o", [C, B_ROIS], f32)

    with tile.TileContext(nc) as tc:
        with (
            tc.tile_pool(name="sm", bufs=2) as sm,
            tc.tile_pool(name="big", bufs=1) as big,
            tc.tile_pool(name="pa", bufs=2, space="PSUM") as pa,
            tc.tile_pool(name="pb", bufs=2, space="PSUM") as pb,
            tc.tile_pool(name="pc", bufs=1, space="PSUM") as pc,
        ):
            ident = big.tile([128, 128], f32)
            make_identity(nc, ident[:])
            epst = big.tile([128, 1], f32)
            nc.vector.memset(epst[:], EPS)
            gs_t = big.tile([128, GOW // 16], i16)
            nc.sync.dma_start(out=gs_t[:], in_=gst[:])
            wu_t = big.tile([128, 1], i16)
            nc.sync.dma_start(out=wu_t[:], in_=wut[:])
            bv_t = big.tile([128, GP], f32)
            nc.sync.dma_start(out=bv_t[:], in_=bvt[:])
            io_t = big.tile([128, 512], f32)
            nc.sync.dma_start(out=io_t[:], in_=iot[:])
            g1t = big.tile([128, 1], f32); nc.sync.dma_start(out=g1t[:], in_=g1[:])
            b1t = big.tile([128, 1], f32); nc.sync.dma_start(out=b1t[:], in_=b1[:])
            g2t = big.tile([128, 1], f32); nc.sync.dma_start(out=g2t[:], in_=g2[:])
            b2t = big.tile([128, 1], f32); nc.sync.dma_start(out=b2t[:], in_=b2[:])
            wst = big.tile([128, C], f32)
            nc.sync.dma_start(out=wst[:], in_=ws[:])

            ft = big.tile([128, NYW], bf16)
            nc.sync.dma_start(out=ft[:], in_=f_p[:])
            w1t = big.tile([128, K3 * C], bf16)
            QW1 = 32 * C
            for q in range(4):
                a0, a1 = q * QW1, min((q + 1) * QW1, K3 * C)
                nc.sync.dma_start(out=w1t[:, a0:a1], in_=w1[:, a0:a1])
            w2t = big.tile([128, GP * C], bf16)
            nc.sync.dma_start(out=w2t[:], in_=w2[:])
            selt = big.tile([128, NSC * XGW], bf16)
            nc.sync.dma_start(out=selt[:], in_=sel[:])
            mskt = big.tile([128, XGW], f32)
            nc.sync.dma_start(out=mskt[:], in_=msk[:])

            # ---------- stage 1: 125 matmuls vs W1 ----------
            xtab = big.tile([128, XTW], f32)
            nc.vector.memset(xtab[:, NYW:XTW], 0.0)
            wrm = big.tile([128, 16 * (NWRM + 1)], f32)
            nc.gpsimd.ap_gather(          # keep DSP pool warm (wake #0)
                out_ap=wrm[:, 0:16].rearrange("p (n u) -> p n u", u=1),
                in_ap=xtab[:, NYW:NYW + 128].rearrange("p (n u) -> p n u", u=1),
                idxs_ap=wu_t[:], channels=128, num_elems=128, d=1, num_idxs=16)
            for ch in range(NCH):
                c0, c1 = ch * 512, min(ch * 512 + 512, NYW)
                yp = pa.tile([128, 512], f32, tag="yp")
                for k in range(c0 // KW, (c1 + KW - 1) // KW):
                    a = max(k * KW, c0)
                    b = min((k + 1) * KW, c1)
                    if a >= b:
                        continue
                    nc.tensor.matmul(out=yp[:, a - c0:b - c0],
                                     lhsT=w1t[:, k * C:(k + 1) * C],
                                     rhs=ft[:, a:b], start=True, stop=True)
                nc.scalar.activation(out=xtab[:, c0:c1],
                                     in_=yp[:, :c1 - c0], func=AF.Copy)
                if ch % 2 == 1 and ch // 2 < NWRM:   # warm-up drumbeat
                    w = ch // 2 + 1
                    nc.gpsimd.ap_gather(
                        out_ap=wrm[:, w * 16:w * 16 + 16].rearrange(
                            "p (n u) -> p n u", u=1),
                        in_ap=xtab[:, c0:c0 + 128].rearrange(
                            "p (n u) -> p n u", u=1),
                        idxs_ap=wu_t[:], channels=128, num_elems=128, d=1,
                        num_idxs=16)

            # ---------- one gather: multi planes + stage-2 slot table ------
            gout = big.tile([128, GOW], f32)
            nc.gpsimd.ap_gather(
                out_ap=gout[:, 0:GOW].rearrange("p (n u) -> p n u", u=1),
                in_ap=xtab[:, 0:XTW].rearrange("p (n u) -> p n u", u=1),
                idxs_ap=gs_t[:], channels=128, num_elems=XTW, d=1,
                num_idxs=GOW)
            off = MW
            for j in range(NRND):
                rw = RWS[j]
                nc.vector.tensor_tensor(out=gout[:, :rw], in0=gout[:, :rw],
                                        in1=gout[:, off:off + rw], op=OP.add)
                off += rw

            # ---------- PE scatter of multi sums into slot table ----------
            acct = big.tile([128, MW], bf16)
            for j in range(NSC):
                tp = pa.tile([128, 128], f32, tag="yp")
                nc.tensor.transpose(out=tp[:],
                                    in_=gout[:, j * 128:(j + 1) * 128],
                                    identity=ident[:])
                nc.vector.tensor_copy(out=acct[:, j * 128:(j + 1) * 128],
                                      in_=tp[:])
            for c0 in range(0, XGW, 512):
                c1 = min(c0 + 512, XGW)
                psc = pb.tile([128, 512], f32, tag="psc")
                for j in range(NSC):
                    nc.tensor.matmul(out=psc[:, :c1 - c0],
                                     lhsT=acct[:, j * 128:(j + 1) * 128],
                                     rhs=selt[:, j * XGW + c0:j * XGW + c1],
                                     start=(j == 0), stop=(j == NSC - 1))
                nc.vector.tensor_tensor(out=gout[:, GB + c0:GB + c1],
                                        in0=gout[:, GB + c0:GB + c1],
                                        in1=psc[:, :c1 - c0], op=OP.add)

            # ---------- masked BN1 stats + AllReduce ----------
            sx = gout[:, GB:GB + XGW]
            r_ = big.tile([128, XGW], f32)
            nc.vector.tensor_tensor(out=r_[:], in0=sx, in1=mskt[:],
                                    op=OP.mult)
            st = big.tile([128, 2], f32)
            nc.vector.reduce_sum(out=st[:, 0:1], in_=r_[:], axis=AX.X)
            nc.scalar.activation(out=r_[:], in_=r_[:], func=AF.Square,
                                 accum_out=st[:, 1:2])
            nc.sync.dma_start(out=cc1i[:], in_=st[:])
            nc.gpsimd.collective_compute(
                "AllReduce", OP.add, replica_groups=[list(range(NCORES))],
                ins=[cc1i[:]], outs=[cc1o[:]])
            stg = big.tile([128, 2], f32)
            nc.sync.dma_start(out=stg[:], in_=cc1o[:])

            # ---------- BN1 constants ----------
            mean = big.tile([128, 1], f32)
            nc.vector.tensor_scalar_mul(out=mean[:], in0=stg[:, 0:1],
                                        scalar1=1.0 / Nq)
            var = big.tile([128, 1], f32)
            nc.vector.tensor_scalar_mul(out=var[:], in0=stg[:, 1:2],
                                        scalar1=1.0 / Nq)
            m2 = big.tile([128, 1], f32)
            nc.vector.tensor_tensor(out=m2[:], in0=mean[:], in1=mean[:],
                                    op=OP.mult)
            nc.vector.tensor_tensor(out=var[:], in0=var[:], in1=m2[:],
                                    op=OP.subtract)
            sd = big.tile([128, 1], f32)
            nc.scalar.activation(out=sd[:], in_=var[:], func=AF.Sqrt,
                                 bias=epst[:, :1])
            rs = big.tile([128, 1], f32)
            nc.vector.reciprocal(out=rs[:], in_=sd[:])
            rsg = big.tile([128, 1], f32)
            nc.vector.tensor_tensor(out=rsg[:], in0=rs[:], in1=g1t[:],
                                    op=OP.mult)
            shift = big.tile([128, 1], f32)
            nc.vector.tensor_tensor(out=shift[:], in0=mean[:], in1=rsg[:],
                                    op=OP.mult)
            nc.vector.tensor_tensor(out=shift[:], in0=b1t[:], in1=shift[:],
                                    op=OP.subtract)
            xz = big.tile([128, 1], f32)
            t1 = big.tile([128, 1], f32)
            nc.scalar.activation(out=xz[:], in_=shift[:], func=AF.Relu)
            nc.vector.tensor_scalar_min(out=t1[:], in0=shift[:], scalar1=0.0)
            nc.scalar.activation(out=t1[:], in_=t1[:], func=AF.Exp)
            nc.vector.tensor_tensor(out=xz[:], in0=xz[:], in1=t1[:], op=OP.add)
            nc.vector.tensor_scalar_add(out=xz[:], in0=xz[:], scalar1=-1.0)
            ccol = big.tile([128, 1], f32)
            nc.vector.tensor_scalar(out=ccol[:], in0=xz[:], scalar1=-1.0,
                                    scalar2=-1.0, op0=OP.mult, op1=OP.add)

            # ---------- BN + ELU on stage-2 cols, minus xz ----------
            nc.vector.tensor_scalar(out=sx, in0=sx, scalar1=rsg[:, :1],
                                    scalar2=shift[:, :1], op0=OP.mult,
                                    op1=OP.add)
            nc.scalar.activation(out=r_[:], in_=sx, func=AF.Relu)
            nc.vector.tensor_scalar_min(out=sx, in0=sx, scalar1=0.0)
            nc.scalar.activation(out=sx, in_=sx, func=AF.Exp)
            nc.vector.tensor_tensor(out=sx, in0=sx, in1=r_[:], op=OP.add)
            nc.vector.tensor_scalar(out=sx, in0=sx, scalar1=ccol[:, :1],
                                    scalar2=None, op0=OP.add)
            sxb = big.tile([128, XGW], bf16)
            nc.vector.tensor_copy(out=sxb[:], in_=sx)

            # ---------- pooling conv: corrections + one-hot aggregation ----
            pool_p = pc.tile([128, 512], f32, tag="pool")
            for q0 in range(0, GP, 4):
                qn = min(4, GP - q0)
                cp = pb.tile([128, 512], f32, tag="psc")
                for lp in range(q0, q0 + qn):
                    nc.tensor.matmul(
                        out=cp[:, (lp - q0) * 128:(lp - q0 + 1) * 128],
                        lhsT=sxb[:, lp * SLOT:(lp + 1) * SLOT],
                        rhs=w2t[:, lp * C:(lp + 1) * C],
                        start=True, stop=True)
                cbf = sm.tile([128, 512], bf16, tag="cbf")
                nc.vector.tensor_copy(out=cbf[:, :qn * 128],
                                      in_=cp[:, :qn * 128])
                for lp in range(q0, q0 + qn):
                    oh = sm.tile([128, 512], bf16, tag="oh")
                    nc.vector.tensor_tensor(
                        out=oh[:],
                        in0=bv_t[:, lp:lp + 1].to_broadcast([128, 512]),
                        in1=io_t[:], op=OP.is_equal)
                    nc.tensor.matmul(
                        out=pool_p[:],
                        lhsT=cbf[:, (lp - q0) * 128:(lp - q0 + 1) * 128],
                        rhs=oh[:], start=(lp == 0), stop=(lp == GP - 1))
            basep = pa.tile([128, 1], f32, tag="yp")
            nc.tensor.matmul(out=basep[:], lhsT=wst[:], rhs=xz[:, :1],
                             start=True, stop=True)
            base = big.tile([128, 1], f32)
            nc.vector.tensor_copy(out=base[:], in_=basep[:])
            pl = big.tile([128, 512], f32)
            nc.vector.tensor_copy(out=pl[:], in_=pool_p[:])
            nc.vector.tensor_scalar(out=pl[:], in0=pl[:], scalar1=base[:, :1],
                                    scalar2=None, op0=OP.add)

            # ---------- pooled AllReduce + final BN ----------
            nc.sync.dma_start(out=cc2i[:], in_=pl[:])
            nc.gpsimd.collective_compute(
                "AllReduce", OP.add, replica_groups=[list(range(NCORES))],
                ins=[cc2i[:]], outs=[cc2o[:]])
            pf = big.tile([128, 512], f32)
            nc.sync.dma_start(out=pf[:], in_=cc2o[:])
            mn2 = big.tile([128, 1], f32)
            nc.vector.reduce_sum(out=mn2[:], in_=pf[:], axis=AX.X)
            nc.vector.tensor_scalar_mul(out=mn2[:], in0=mn2[:],
                                        scalar1=1.0 / B_ROIS)
            sq2 = big.tile([128, 1], f32)
            scr2 = big.tile([128, 512], f32)
            nc.scalar.activation(out=scr2[:], in_=pf[:], func=AF.Square,
                                 accum_out=sq2[:])
            nc.vector.tensor_scalar_mul(out=sq2[:], in0=sq2[:],
                                        scalar1=1.0 / B_ROIS)
            m22 = big.tile([128, 1], f32)
            nc.vector.tensor_tensor(out=m22[:], in0=mn2[:], in1=mn2[:],
                                    op=OP.mult)
            nc.vector.tensor_tensor(out=sq2[:], in0=sq2[:], in1=m22[:],
                                    op=OP.subtract)
            sd2 = big.tile([128, 1], f32)
            nc.scalar.activation(out=sd2[:], in_=sq2[:], func=AF.Sqrt,
                                 bias=epst[:, :1])
            rs2 = big.tile([128, 1], f32)
            nc.vector.reciprocal(out=rs2[:], in_=sd2[:])
            rsg2 = big.tile([128, 1], f32)
            nc.vector.tensor_tensor(out=rsg2[:], in0=rs2[:], in1=g2t[:],
                                    op=OP.mult)
            sh2 = big.tile([128, 1], f32)
            nc.vector.tensor_tensor(out=sh2[:], in0=mn2[:], in1=rsg2[:],
                                    op=OP.mult)
            nc.vector.tensor_tensor(out=sh2[:], in0=b2t[:], in1=sh2[:],
                                    op=OP.subtract)
            nc.vector.tensor_scalar(out=pf[:], in0=pf[:], scalar1=rsg2[:, :1],
                                    scalar2=sh2[:, :1], op0=OP.mult,
                                    op1=OP.add)
            nc.sync.dma_start(out=out[:], in_=pf[:])

    nc.compile()
    return nc


def _build_inputs(cores, S, sp_feats, W1, W2, gamma1, beta1, gamma2, beta2):
    NYW, GOW, NSC, XGW = S["NYW"], S["GOW"], S["NSC"], S["XGW"]
    W1t = np.ascontiguousarray(
        W1.transpose(1, 0, 2).reshape(C, K3 * C)).astype(BF16)
    base_in = {
        "w1": W1t,
        "g1": gamma1.reshape(C, 1), "b1": beta1.reshape(C, 1),
        "g2": gamma2.reshape(C, 1), "b2": beta2.reshape(C, 1),
        "iot": np.broadcast_to(np.arange(512, dtype=np.float32),
                               (128, 512)).copy(),
        "wut": np.zeros((128, 1), np.int16),
    }
    in_maps = []
    for c in range(NCORES):
        L = cores[c]
        m = dict(base_in)
        F = np.zeros((C, NYW), np.float32)
        F[:, L["fcols"]] = sp_feats[L["frows"]].T
        m["f"] = F.astype(BF16)
        w2l = np.zeros((GP, C, C), np.float32)
        p0 = c * GP
        nreal = max(0, min(GP, P - p0))
        w2l[:nreal] = W2[p0:p0 + nreal]
        m["ws"] = np.ascontiguousarray(w2l.sum(0))
        m["w2"] = np.ascontiguousarray(
            w2l.transpose(1, 0, 2).reshape(C, GP * C)).astype(BF16)
        m["gst"] = _wrap16(L["gidx"], GOW)
        # selm [MW, XGW] -> [128, NSC*XGW] (source chunk j in partitions)
        selw = np.zeros((128, NSC * XGW), np.float32)
        for j in range(NSC):
            selw[:, j * XGW:(j + 1) * XGW] = L["selm"][j * 128:(j + 1) * 128]
        m["sel"] = selw.astype(BF16)
        m["msk"] = np.broadcast_to(L["msk"], (C, XGW)).copy()
        bvw = np.zeros((128, GP), np.float32)
        for lp in range(GP):
            bvw[:, lp] = L["bv"][lp * SLOT:(lp + 1) * SLOT]
        m["bvt"] = bvw
        in_maps.append(m)
    return in_maps


def kernel(**inputs):
    sp_coords = np.asarray(inputs["sp_coords"])
    sp_feats = np.asarray(inputs["sp_feats"], np.float32)
    grid_points = np.asarray(inputs["grid_points"], np.float32)
    W1 = np.asarray(inputs["W1"], np.float32)
    gamma1 = np.asarray(inputs["gamma1"], np.float32)
    beta1 = np.asarray(inputs["beta1"], np.float32)
    W2 = np.asarray(inputs["W2"], np.float32)
    gamma2 = np.asarray(inputs["gamma2"], np.float32)
    beta2 = np.asarray(inputs["beta2"], np.float32)

    Nq, unq_inv, hit_rows, hit_ks, hit_sp = _host_indices(sp_coords,
                                                          grid_points)
    cores, S = _shard(Nq, unq_inv, hit_rows, hit_ks, hit_sp)

    key = tuple(sorted((k, v) for k, v in S.items() if k != "RWS")) \
        + S["RWS"]
    if key not in _cache:
        _cache.clear()
        _cache[key] = _compile(S)
    nc = _cache[key]

    in_maps = _build_inputs(cores, S, sp_feats, W1, W2, gamma1, beta1,
                            gamma2, beta2)

    import os
    from concourse.bass_utils import run_bass_kernel_spmd
    trace = os.environ.get("KERNEL_TRACE", "0") == "1"
    if trace:
        try:
            import ntff_hook
            ntff_hook.install()
        except Exception:
            trace = False
    res = run_bass_kernel_spmd(nc, in_maps, list(range(NCORES)), trace=trace)
    if trace and res.exec_time_ns:
        print("HW exec time: %d ns" % res.exec_time_ns)
    return np.ascontiguousarray(
        np.asarray(res.results[0]["out"], np.float32).T)


# revision 8
# speedup vs baseline: 1.4354x; 1.3587x over previous
"""Trainium2 Bass kernel for nn_CAGroup3DRoIHead (sparse conv + BN + ELU +
grid pooling + BN), 8-core SPMD.

Sharding: stage 1 (sparse conv) and stage 2 (grid pooling) are both
sharded by pooling cell p (43 cells per core); each core only processes
the unique voxels its cells reference.  Host does all integer index
math and pre-gathers sp_feats rows into a k-major padded table per core
(pure data movement); device does all float compute: 125 matmuls vs W1
(bf16), one SBUF gather (multi-hit planes + stage-2 slot table), a PE
one-hot scatter of multi-hit sums, masked global-BN stats + tiny
AllReduce, BN+ELU, pooling conv vs W2 (bf16, one-hot matmul aggregation
over ROIs), pooled AllReduce, final BN.
"""
import numpy as np
import ml_dtypes

G = 384
HALF = G // 2
SZ = G
SYZ = G * G
SXYZ = G * G * G
VOX = np.float32(0.08)
CK = 2
K = 5
K3 = K ** 3
GN = 7
EPS = 1e-5
C = 128
NCORES = 8
NV = 200000
N = 175616
P = GN ** 3          # 343
B_ROIS = 512
GP = 43              # p-cells per core (core 7: 42 real + 1 pad)
SLOT = 64            # padded slots per p-cell (max real = 62)

BF16 = ml_dtypes.bfloat16
_cache = {}


def _pad(n, m):
    return ((int(n) + m - 1) // m) * m


def _wrap16(lst, pad_to):
    lst = np.asarray(lst, np.int64)
    n = len(lst)
    s = (pad_to + 15) // 16
    out = np.zeros((128, s), np.int16)
    padded = np.concatenate([lst, np.zeros(pad_to - n, np.int64)])
    for r in range(8):
        for p in range(16):
            row = padded[p::16]
            out[16 * r + p, :len(row)] = row
    return out


def _host_indices(sp_coords, grid_points):
    """Replicates reference.py's integer index math exactly."""
    sp_coords = np.asarray(sp_coords)
    grid_points = np.asarray(grid_points, np.float32)
    vox = np.clip(np.floor(grid_points[:, 1:4] / VOX).astype(np.int32),
                  -(HALF - 1), HALF - 1).astype(np.int64)
    pos = vox + HALF
    bidx = grid_points[:, 0].astype(np.int64)
    mc = bidx * SXYZ + pos[:, 0] * SYZ + pos[:, 1] * SZ + pos[:, 2]
    unq, unq_inv = np.unique(mc, return_inverse=True)
    Nq = len(unq)
    qb = unq // SXYZ
    qv = np.stack([unq % SXYZ // SYZ, unq % SYZ // SZ, unq % SZ], 1)

    svi = sp_coords[:, 1:4].astype(np.int64) // CK + HALF
    scode = sp_coords[:, 0].astype(np.int64) * SXYZ + svi[:, 0] * SYZ \
        + svi[:, 1] * SZ + svi[:, 2]
    order = np.argsort(scode, kind="stable")
    scodes = scode[order]

    kr = np.arange(-(K // 2), K // 2 + 1)
    offs = np.stack(np.meshgrid(kr, kr, kr, indexing="ij"), -1).reshape(-1, 3)
    hit_rows, hit_ks, hit_sp = [], [], []
    for k in range(K3):
        tvi = qv + offs[k]
        inb = np.all((tvi >= 0) & (tvi < G), axis=1)
        code = qb * SXYZ + tvi[:, 0] * SYZ + tvi[:, 1] * SZ + tvi[:, 2]
        pp = np.clip(np.searchsorted(scodes, code), 0, NV - 1)
        hit = (scodes[pp] == code) & inb
        w = np.nonzero(hit)[0]
        hit_rows.append(w)
        hit_ks.append(np.full(len(w), k, np.int64))
        hit_sp.append(order[pp[w]])
    hit_rows = np.concatenate(hit_rows)   # voxel slot of hit, k-major order
    hit_ks = np.concatenate(hit_ks)
    hit_sp = np.concatenate(hit_sp)
    return Nq, unq_inv, hit_rows, hit_ks, hit_sp


def _shard(Nq, unq_inv, hit_rows, hit_ks, hit_sp):
    """Partition work by pooling cell p; build per-core index tables."""
    pp = np.arange(N) % P
    bb = np.arange(N) // P
    core_slot = np.minimum(pp // GP, NCORES - 1)
    order_v = np.argsort(hit_rows, kind="stable")
    row_of = hit_rows[order_v]
    has_hit = np.zeros(Nq, bool)
    has_hit[hit_rows] = True
    owner = np.full(Nq, NCORES, np.int64)
    for c in range(NCORES - 1, -1, -1):
        owner[np.unique(unq_inv[core_slot == c])] = c

    raw = []
    for c in range(NCORES):
        slots_c = np.nonzero((core_slot == c) & has_hit[unq_inv])[0]
        vox_c = np.unique(unq_inv[slots_c])          # sorted, all have hits
        a = np.searchsorted(row_of, vox_c, "left")
        b = np.searchsorted(row_of, vox_c, "right")
        cnt = b - a
        all_h = np.concatenate([order_v[x:y] for x, y in zip(a, b)]) \
            if len(vox_c) else np.zeros(0, np.int64)
        raw.append((slots_c, vox_c, a, cnt, all_h))

    # shared padded sizes (max over cores)
    KW = 4
    maxcnt = 2
    max_multi = 1
    for c in range(NCORES):
        slots_c, vox_c, a, cnt, all_h = raw[c]
        kc = np.bincount(hit_ks[all_h], minlength=K3)
        KW = max(KW, _pad(kc.max(), 4))
        maxcnt = max(maxcnt, int(cnt.max()))
        max_multi = max(max_multi, int((cnt >= 2).sum()))
    NYW = K3 * KW
    MW = _pad(max_multi, 128)                        # scatter-source chunks
    NSC = MW // 128
    NRND = maxcnt - 1
    RWS = []
    for j in range(1, maxcnt):
        m = max(max(int((r[3] > j).sum()) for r in raw), 1)
        RWS.append(_pad(m, 4))
    GRW = _pad(sum(RWS), 16)
    ZC = NYW                                         # zero col, Y-relative
    XGW = GP * SLOT
    GB = MW + GRW                                    # xg offset in gout
    GOW = GB + XGW                                   # gather output width

    cores = []
    for c in range(NCORES):
        slots_c, vox_c, a, cnt, all_h = raw[c]
        ks_h = hit_ks[all_h]
        sp_h = hit_sp[all_h]
        ord_k = np.argsort(ks_h, kind="stable")
        ks_sorted = ks_h[ord_k]
        kcounts = np.bincount(ks_sorted, minlength=K3)
        kstart = np.concatenate([[0], np.cumsum(kcounts)])[:-1]
        rank = np.arange(len(ks_sorted)) - kstart[ks_sorted]
        col_of_hit = np.empty(len(all_h), np.int64)
        col_of_hit[ord_k] = ks_sorted * KW + rank
        starts = np.concatenate([[0], np.cumsum(cnt)])
        first_col = col_of_hit[starts[:-1]]

        multi_mask = cnt >= 2
        m_idx = np.nonzero(multi_mask)[0]
        m_ord = np.argsort(-cnt[multi_mask], kind="stable")
        multi_vloc = m_idx[m_ord]                    # count-desc local vox ids
        n_multi = len(multi_vloc)
        mcnt = cnt[multi_vloc]
        pos_in_accr = np.full(len(vox_c), -1, np.int64)
        pos_in_accr[multi_vloc] = np.arange(n_multi)

        # gather idx: [P0 hit1 of multis | planes hits 2..6 | xg slots]
        gidx = np.full(GOW, ZC, np.int64)
        gidx[:n_multi] = first_col[multi_vloc]
        off = MW
        for j in range(1, maxcnt):
            sel = multi_vloc[mcnt > j]               # prefix (count-desc)
            nj = len(sel)
            gidx[off:off + nj] = col_of_hit[starts[sel] + j]
            off += RWS[j - 1]

        bv = np.full(XGW, 600.0, np.float32)
        selm = np.zeros((MW, XGW), np.float32)       # scatter one-hot
        msk = np.zeros(XGW, np.float32)              # owned stats mask
        own = owner[vox_c] == c
        seen = np.zeros(len(vox_c), bool)
        vox_of_slot = unq_inv[slots_c]
        p_of_slot = pp[slots_c]
        for lp in range(GP):
            p = c * GP + lp
            if p >= P:
                continue
            m = p_of_slot == p
            sl = slots_c[m]
            n = len(sl)
            assert n <= SLOT
            vl = np.searchsorted(vox_c, vox_of_slot[m])
            dst = GB + lp * SLOT + np.arange(n)
            is_m = pos_in_accr[vl] >= 0
            gidx[dst[~is_m]] = first_col[vl[~is_m]]  # singles: direct col
            selm[pos_in_accr[vl[is_m]], dst[is_m] - GB] = 1.0
            newly = own[vl] & ~seen[vl]
            msk[dst[newly] - GB] = 1.0
            seen[vl[newly]] = True
            bv[lp * SLOT: lp * SLOT + n] = bb[sl]
        cores.append(dict(fcols=col_of_hit, frows=sp_h, gidx=gidx, bv=bv,
                          selm=selm, msk=msk, n_multi=n_multi))
    sizes = dict(KW=KW, NYW=NYW, MW=MW, NSC=NSC, NRND=NRND, RWS=tuple(RWS),
                 GRW=GRW, ZC=ZC, XGW=XGW, GB=GB, GOW=GOW, Nq=Nq)
    return cores, sizes


def _compile(S):
    import concourse.bass as bass
    import concourse.bacc as bacc
    import concourse.tile as tile
    from concourse import mybir
    from concourse.masks import make_identity

    f32 = mybir.dt.float32
    bf16 = mybir.dt.bfloat16
    i16 = mybir.dt.int16
    AF = mybir.ActivationFunctionType
    OP = mybir.AluOpType
    AX = mybir.AxisListType

    KW, NYW, MW, NSC, NRND = S["KW"], S["NYW"], S["MW"], S["NSC"], S["NRND"]
    RWS, GRW, XGW, GB, GOW = S["RWS"], S["GRW"], S["XGW"], S["GB"], S["GOW"]
    Nq = float(S["Nq"])
    XTW = NYW + 128
    NCH = (NYW + 511) // 512
    NWRM = 4                                         # warm-up dummy gathers

    nc = bacc.Bacc("TRN2", target_bir_lowering=False, debug=False,
                   num_devices=NCORES, num_swdge_queues=1)
    f_p = nc.declare_dram_parameter("f", [C, NYW], bf16, isOutput=False)
    w1 = nc.declare_dram_parameter("w1", [C, K3 * C], bf16, isOutput=False)
    w2 = nc.declare_dram_parameter("w2", [C, GP * C], bf16, isOutput=False)
    ws = nc.declare_dram_parameter("ws", [C, C], f32, isOutput=False)
    sel = nc.declare_dram_parameter("sel", [C, NSC * XGW], bf16,
                                    isOutput=False)
    msk = nc.declare_dram_parameter("msk", [C, XGW], f32, isOutput=False)
    g1 = nc.declare_dram_parameter("g1", [C, 1], f32, isOutput=False)
    b1 = nc.declare_dram_parameter("b1", [C, 1], f32, isOutput=False)
    g2 = nc.declare_dram_parameter("g2", [C, 1], f32, isOutput=False)
    b2 = nc.declare_dram_parameter("b2", [C, 1], f32, isOutput=False)
    gst = nc.declare_dram_parameter("gst", [128, GOW // 16], i16,
                                    isOutput=False)
    wut = nc.declare_dram_parameter("wut", [128, 1], i16, isOutput=False)
    bvt = nc.declare_dram_parameter("bvt", [128, GP], f32, isOutput=False)
    iot = nc.declare_dram_parameter("iot", [128, 512], f32, isOutput=False)
    out = nc.declare_dram_parameter("out", [C, B_ROIS], f32, isOutput=True)
    cc1i = nc.dram_tensor("cc1i", [C, 2], f32)
    cc1o = nc.dram_tensor("cc1o", [C, 2], f32)
    cc2i = nc.dram_tensor("cc2i", [C, B_ROIS], f32)
    cc2o = nc.dram_tensor("cc2o", [C, B_ROIS], f32)

    with tile.TileContext(nc) as tc:
        with (
            tc.tile_pool(name="sm", bufs=2) as sm,
            tc.tile_pool(name="big", bufs=1) as big,
            tc.tile_pool(name="pa", bufs=2, space="PSUM") as pa,
            tc.tile_pool(name="pb", bufs=2, space="PSUM") as pb,
            tc.tile_pool(name="pc", bufs=1, space="PSUM") as pc,
        ):
            ident = big.tile([128, 128], f32)
            make_identity(nc, ident[:])
            epst = big.tile([128, 1], f32)
            nc.vector.memset(epst[:], EPS)
            gs_t = big.tile([128, GOW // 16], i16)
            nc.sync.dma_start(out=gs_t[:], in_=gst[:])
            wu_t = big.tile([128, 1], i16)
            nc.sync.dma_start(out=wu_t[:], in_=wut[:])
            bv_t = big.tile([128, GP], f32)
            nc.sync.dma_start(out=bv_t[:], in_=bvt[:])
            io_t = big.tile([128, 512], f32)
            nc.sync.dma_start(out=io_t[:], in_=iot[:])
            g1t = big.tile([128, 1], f32); nc.sync.dma_start(out=g1t[:], in_=g1[:])
            b1t = big.tile([128, 1], f32); nc.sync.dma_start(out=b1t[:], in_=b1[:])
            g2t = big.tile([128, 1], f32); nc.sync.dma_start(out=g2t[:], in_=g2[:])
            b2t = big.tile([128, 1], f32); nc.sync.dma_start(out=b2t[:], in_=b2[:])
            wst = big.tile([128, C], f32)
            nc.sync.dma_start(out=wst[:], in_=ws[:])

            ft = big.tile([128, NYW], bf16)
            nc.sync.dma_start(out=ft[:], in_=f_p[:])
            w1t = big.tile([128, K3 * C], bf16)
            QW1 = 32 * C
            for q in range(4):
                a0, a1 = q * QW1, min((q + 1) * QW1, K3 * C)
                nc.sync.dma_start(out=w1t[:, a0:a1], in_=w1[:, a0:a1])
            w2t = big.tile([128, GP * C], bf16)
            nc.sync.dma_start(out=w2t[:], in_=w2[:])
            selt = big.tile([128, NSC * XGW], bf16)
            nc.sync.dma_start(out=selt[:], in_=sel[:])
            mskt = big.tile([128, XGW], f32)
            nc.sync.dma_start(out=mskt[:], in_=msk[:])

            # ---------- stage 1: 125 matmuls vs W1 ----------
            xtab = big.tile([128, XTW], f32)
            nc.vector.memset(xtab[:, NYW:XTW], 0.0)
            wrm = big.tile([128, 16 * (NWRM + 1)], f32)
            nc.gpsimd.ap_gather(          # keep DSP pool warm (wake #0)
                out_ap=wrm[:, 0:16].rearrange("p (n u) -> p n u", u=1),
                in_ap=xtab[:, NYW:NYW + 128].rearrange("p (n u) -> p n u", u=1),
                idxs_ap=wu_t[:], channels=128, num_elems=128, d=1, num_idxs=16)
            for ch in range(NCH):
                c0, c1 = ch * 512, min(ch * 512 + 512, NYW)
                yp = pa.tile([128, 512], f32, tag="yp")
                for k in range(c0 // KW, (c1 + KW - 1) // KW):
                    a = max(k * KW, c0)
                    b = min((k + 1) * KW, c1)
                    if a >= b:
                        continue
                    nc.tensor.matmul(out=yp[:, a - c0:b - c0],
                                     lhsT=w1t[:, k * C:(k + 1) * C],
                                     rhs=ft[:, a:b], start=True, stop=True)
                nc.scalar.activation(out=xtab[:, c0:c1],
                                     in_=yp[:, :c1 - c0], func=AF.Copy)
                if ch % 2 == 1 and ch // 2 < NWRM:   # warm-up drumbeat
                    w = ch // 2 + 1
                    nc.gpsimd.ap_gather(
                        out_ap=wrm[:, w * 16:w * 16 + 16].rearrange(
                            "p (n u) -> p n u", u=1),
                        in_ap=xtab[:, c0:c0 + 128].rearrange(
                            "p (n u) -> p n u", u=1),
                        idxs_ap=wu_t[:], channels=128, num_elems=128, d=1,
                        num_idxs=16)

            # ---------- one gather: multi planes + stage-2 slot table ------
            gout = big.tile([128, GOW], f32)
            nc.gpsimd.ap_gather(
                out_ap=gout[:, 0:GOW].rearrange("p (n u) -> p n u", u=1),
                in_ap=xtab[:, 0:XTW].rearrange("p (n u) -> p n u", u=1),
                idxs_ap=gs_t[:], channels=128, num_elems=XTW, d=1,
                num_idxs=GOW)
            off = MW
            for j in range(NRND):
                rw = RWS[j]
                nc.vector.tensor_tensor(out=gout[:, :rw], in0=gout[:, :rw],
                                        in1=gout[:, off:off + rw], op=OP.add)
                off += rw

            # ---------- PE scatter of multi sums into slot table ----------
            acct = big.tile([128, MW], bf16)
            for j in range(NSC):
                tp = pa.tile([128, 128], f32, tag="yp")
                nc.tensor.transpose(out=tp[:],
                                    in_=gout[:, j * 128:(j + 1) * 128],
                                    identity=ident[:])
                nc.vector.tensor_copy(out=acct[:, j * 128:(j + 1) * 128],
                                      in_=tp[:])
            for c0 in range(0, XGW, 512):
                c1 = min(c0 + 512, XGW)
                psc = pb.tile([128, 512], f32, tag="psc")
                for j in range(NSC):
                    nc.tensor.matmul(out=psc[:, :c1 - c0],
                                     lhsT=acct[:, j * 128:(j + 1) * 128],
                                     rhs=selt[:, j * XGW + c0:j * XGW + c1],
                                     start=(j == 0), stop=(j == NSC - 1))
                nc.vector.tensor_tensor(out=gout[:, GB + c0:GB + c1],
                                        in0=gout[:, GB + c0:GB + c1],
                                        in1=psc[:, :c1 - c0], op=OP.add)

            # ---------- masked BN1 stats + AllReduce ----------
            sx = gout[:, GB:GB + XGW]
            r_ = big.tile([128, XGW], f32)
            nc.vector.tensor_tensor(out=r_[:], in0=sx, in1=mskt[:],
                                    op=OP.mult)
            st = big.tile([128, 2], f32)
            nc.vector.reduce_sum(out=st[:, 0:1], in_=r_[:], axis=AX.X)
            nc.scalar.activation(out=r_[:], in_=r_[:], func=AF.Square,
                                 accum_out=st[:, 1:2])
            nc.sync.dma_start(out=cc1i[:], in_=st[:])
            nc.gpsimd.collective_compute(
                "AllReduce", OP.add, replica_groups=[list(range(NCORES))],
                ins=[cc1i[:]], outs=[cc1o[:]])
            stg = big.tile([128, 2], f32)
            nc.sync.dma_start(out=stg[:], in_=cc1o[:])

            # ---------- BN1 constants ----------
            mean = big.tile([128, 1], f32)
            nc.vector.tensor_scalar_mul(out=mean[:], in0=stg[:, 0:1],
                                        scalar1=1.0 / Nq)
            var = big.tile([128, 1], f32)
            nc.vector.tensor_scalar_mul(out=var[:], in0=stg[:, 1:2],
                                        scalar1=1.0 / Nq)
            m2 = big.tile([128, 1], f32)
            nc.vector.tensor_tensor(out=m2[:], in0=mean[:], in1=mean[:],
                                    op=OP.mult)
            nc.vector.tensor_tensor(out=var[:], in0=var[:], in1=m2[:],
                                    op=OP.subtract)
            sd = big.tile([128, 1], f32)
            nc.scalar.activation(out=sd[:], in_=var[:], func=AF.Sqrt,
                                 bias=epst[:, :1])
            rs = big.tile([128, 1], f32)
            nc.vector.reciprocal(out=rs[:], in_=sd[:])
            rsg = big.tile([128, 1], f32)
            nc.vector.tensor_tensor(out=rsg[:], in0=rs[:], in1=g1t[:],
                                    op=OP.mult)
            shift = big.tile([128, 1], f32)
            nc.vector.tensor_tensor(out=shift[:], in0=mean[:], in1=rsg[:],
                                    op=OP.mult)
            nc.vector.tensor_tensor(out=shift[:], in0=b1t[:], in1=shift[:],
                                    op=OP.subtract)
            xz = big.tile([128, 1], f32)
            t1 = big.tile([128, 1], f32)
            nc.scalar.activation(out=xz[:], in_=shift[:], func=AF.Relu)
            nc.vector.tensor_scalar_min(out=t1[:], in0=shift[:], scalar1=0.0)
            nc.scalar.activation(out=t1[:], in_=t1[:], func=AF.Exp)
            nc.vector.tensor_tensor(out=xz[:], in0=xz[:], in1=t1[:], op=OP.add)
            nc.vector.tensor_scalar_add(out=xz[:], in0=xz[:], scalar1=-1.0)
            ccol = big.tile([128, 1], f32)
            nc.vector.tensor_scalar(out=ccol[:], in0=xz[:], scalar1=-1.0,
                                    scalar2=-1.0, op0=OP.mult, op1=OP.add)

            # ---------- BN + ELU on stage-2 cols, minus xz ----------
            nc.vector.tensor_scalar(out=sx, in0=sx, scalar1=rsg[:, :1],
                                    scalar2=shift[:, :1], op0=OP.mult,
                                    op1=OP.add)
            nc.scalar.activation(out=r_[:], in_=sx, func=AF.Relu)
            nc.vector.tensor_scalar_min(out=sx, in0=sx, scalar1=0.0)
            nc.scalar.activation(out=sx, in_=sx, func=AF.Exp)
            nc.vector.tensor_tensor(out=sx, in0=sx, in1=r_[:], op=OP.add)
            nc.vector.tensor_scalar(out=sx, in0=sx, scalar1=ccol[:, :1],
                                    scalar2=None, op0=OP.add)
            sxb = big.tile([128, XGW], bf16)
            nc.vector.tensor_copy(out=sxb[:], in_=sx)

            # ---------- pooling conv: corrections + one-hot aggregation ----
            pool_p = pc.tile([128, 512], f32, tag="pool")
            for q0 in range(0, GP, 4):
                qn = min(4, GP - q0)
                cp = pb.tile([128, 512], f32, tag="psc")
                for lp in range(q0, q0 + qn):
                    nc.tensor.matmul(
                        out=cp[:SLOT, (lp - q0) * 128:(lp - q0 + 1) * 128],
                        lhsT=sxb[:, lp * SLOT:(lp + 1) * SLOT],
                        rhs=w2t[:, lp * C:(lp + 1) * C],
                        start=True, stop=True)
                cbf = sm.tile([128, 512], bf16, tag="cbf")
                nc.vector.tensor_copy(out=cbf[:SLOT, :qn * 128],
                                      in_=cp[:SLOT, :qn * 128])
                for lp in range(q0, q0 + qn):
                    oh = sm.tile([128, 512], bf16, tag="oh")
                    nc.vector.tensor_tensor(
                        out=oh[:SLOT, :],
                        in0=bv_t[:SLOT, lp:lp + 1].to_broadcast([SLOT, 512]),
                        in1=io_t[:SLOT, :], op=OP.is_equal)
                    nc.tensor.matmul(
                        out=pool_p[:],
                        lhsT=cbf[:SLOT, (lp - q0) * 128:(lp - q0 + 1) * 128],
                        rhs=oh[:SLOT, :], start=(lp == 0),
                        stop=(lp == GP - 1))
            basep = pa.tile([128, 1], f32, tag="yp")
            nc.tensor.matmul(out=basep[:], lhsT=wst[:], rhs=xz[:, :1],
                             start=True, stop=True)
            base = big.tile([128, 1], f32)
            nc.vector.tensor_copy(out=base[:], in_=basep[:])
            pl = big.tile([128, 512], f32)
            nc.vector.tensor_copy(out=pl[:], in_=pool_p[:])
            nc.vector.tensor_scalar(out=pl[:], in0=pl[:], scalar1=base[:, :1],
                                    scalar2=None, op0=OP.add)

            # ---------- pooled AllReduce + final BN ----------
            nc.sync.dma_start(out=cc2i[:], in_=pl[:])
            nc.gpsimd.collective_compute(
                "AllReduce", OP.add, replica_groups=[list(range(NCORES))],
                ins=[cc2i[:]], outs=[cc2o[:]])
            pf = big.tile([128, 512], f32)
            nc.sync.dma_start(out=pf[:], in_=cc2o[:])
            mn2 = big.tile([128, 1], f32)
            nc.vector.reduce_sum(out=mn2[:], in_=pf[:], axis=AX.X)
            nc.vector.tensor_scalar_mul(out=mn2[:], in0=mn2[:],
                                        scalar1=1.0 / B_ROIS)
            sq2 = big.tile([128, 1], f32)
            scr2 = big.tile([128, 512], f32)
            nc.scalar.activation(out=scr2[:], in_=pf[:], func=AF.Square,
                                 accum_out=sq2[:])
            nc.vector.tensor_scalar_mul(out=sq2[:], in0=sq2[:],
                                        scalar1=1.0 / B_ROIS)
            m22 = big.tile([128, 1], f32)
            nc.vector.tensor_tensor(out=m22[:], in0=mn2[:], in1=mn2[:],
                                    op=OP.mult)
            nc.vector.tensor_tensor(out=sq2[:], in0=sq2[:], in1=m22[:],
                                    op=OP.subtract)
            sd2 = big.tile([128, 1], f32)
            nc.scalar.activation(out=sd2[:], in_=sq2[:], func=AF.Sqrt,
                                 bias=epst[:, :1])
            rs2 = big.tile([128, 1], f32)
            nc.vector.reciprocal(out=rs2[:], in_=sd2[:])
            rsg2 = big.tile([128, 1], f32)
            nc.vector.tensor_tensor(out=rsg2[:], in0=rs2[:], in1=g2t[:],
                                    op=OP.mult)
            sh2 = big.tile([128, 1], f32)
            nc.vector.tensor_tensor(out=sh2[:], in0=mn2[:], in1=rsg2[:],
                                    op=OP.mult)
            nc.vector.tensor_tensor(out=sh2[:], in0=b2t[:], in1=sh2[:],
                                    op=OP.subtract)
            nc.vector.tensor_scalar(out=pf[:], in0=pf[:], scalar1=rsg2[:, :1],
                                    scalar2=sh2[:, :1], op0=OP.mult,
                                    op1=OP.add)
            nc.sync.dma_start(out=out[:], in_=pf[:])

    nc.compile()
    return nc


def _build_inputs(cores, S, sp_feats, W1, W2, gamma1, beta1, gamma2, beta2):
    NYW, GOW, NSC, XGW = S["NYW"], S["GOW"], S["NSC"], S["XGW"]
    W1t = np.ascontiguousarray(
        W1.transpose(1, 0, 2).reshape(C, K3 * C)).astype(BF16)
    base_in = {
        "w1": W1t,
        "g1": gamma1.reshape(C, 1), "b1": beta1.reshape(C, 1),
        "g2": gamma2.reshape(C, 1), "b2": beta2.reshape(C, 1),
        "iot": np.broadcast_to(np.arange(512, dtype=np.float32),
                               (128, 512)).copy(),
        "wut": np.zeros((128, 1), np.int16),
    }
    in_maps = []
    for c in range(NCORES):
        L = cores[c]
        m = dict(base_in)
        F = np.zeros((C, NYW), np.float32)
        F[:, L["fcols"]] = sp_feats[L["frows"]].T
        m["f"] = F.astype(BF16)
        w2l = np.zeros((GP, C, C), np.float32)
        p0 = c * GP
        nreal = max(0, min(GP, P - p0))
        w2l[:nreal] = W2[p0:p0 + nreal]
        m["ws"] = np.ascontiguousarray(w2l.sum(0))
        m["w2"] = np.ascontiguousarray(
            w2l.transpose(1, 0, 2).reshape(C, GP * C)).astype(BF16)
        m["gst"] = _wrap16(L["gidx"], GOW)
        # selm [MW, XGW] -> [128, NSC*XGW] (source chunk j in partitions)
        selw = np.zeros((128, NSC * XGW), np.float32)
        for j in range(NSC):
            selw[:, j * XGW:(j + 1) * XGW] = L["selm"][j * 128:(j + 1) * 128]
        m["sel"] = selw.astype(BF16)
        m["msk"] = np.broadcast_to(L["msk"], (C, XGW)).copy()
        bvw = np.zeros((128, GP), np.float32)
        for lp in range(GP):
            bvw[:SLOT, lp] = L["bv"][lp * SLOT:(lp + 1) * SLOT]
        m["bvt"] = bvw
        in_maps.append(m)
    return in_maps


def kernel(**inputs):
    sp_coords = np.asarray(inputs["sp_coords"])
    sp_feats = np.asarray(inputs["sp_feats"], np.float32)
    grid_points = np.asarray(inputs["grid_points"], np.float32)
    W1 = np.asarray(inputs["W1"], np.float32)
    gamma1 = np.asarray(inputs["gamma1"], np.float32)
    beta1 = np.asarray(inputs["beta1"], np.float32)
    W2 = np.asarray(inputs["W2"], np.float32)
    gamma2 = np.asarray(inputs["gamma2"], np.float32)
    beta2 = np.asarray(inputs["beta2"], np.float32)

    Nq, unq_inv, hit_rows, hit_ks, hit_sp = _host_indices(sp_coords,
                                                          grid_points)
    cores, S = _shard(Nq, unq_inv, hit_rows, hit_ks, hit_sp)

    key = tuple(sorted((k, v) for k, v in S.items() if k != "RWS")) \
        + S["RWS"]
    if key not in _cache:
        _cache.clear()
        _cache[key] = _compile(S)
    nc = _cache[key]

    in_maps = _build_inputs(cores, S, sp_feats, W1, W2, gamma1, beta1,
                            gamma2, beta2)

    import os
    from concourse.bass_utils import run_bass_kernel_spmd
    trace = os.environ.get("KERNEL_TRACE", "0") == "1"
    if trace:
        try:
            import ntff_hook
            ntff_hook.install()
        except Exception:
            trace = False
    res = run_bass_kernel_spmd(nc, in_maps, list(range(NCORES)), trace=trace)
    if trace and res.exec_time_ns:
        print("HW exec time: %d ns" % res.exec_time_ns)
    return np.ascontiguousarray(
        np.asarray(res.results[0]["out"], np.float32).T)
